# revision 14
# baseline (speedup 1.0000x reference)
"""Trainium2 Bass kernel for nn_Decoder_14121852469560.

Pointer-generator decoder step: LSTMCell + intra-temporal (encoder) attention +
intra-decoder attention + vocab softmax + pointer mix + scatter-add of copy
probabilities.

Sharding (8 NeuronCores):
  * batch B=64 data-parallel (8 rows/core) for the LSTM + both attentions,
  * vocab V=50000 tensor-parallel (6250 cols/core) for the 3H x V projection,
  * one AllGather of feats.T [3072, 8] -> [8*3072, 8], one AllReduce of the
    softmax denominators [64].
Everything is self-contained: shapes/sharding hardcoded, host-side numpy does
input prep (transposes/casts/shards), device does all the FLOPs, host does the
final gather + tiny scatter-add.
"""

import numpy as np

import concourse.bass as bass
import concourse.mybir as mybir
import concourse.tile as tile
from concourse import bacc
from concourse.bass_utils import run_bass_kernel_spmd
from concourse.masks import make_identity

dt = mybir.dt

# ---------------- problem constants ----------------
NC = 8
B, E, H, S, T, V = 64, 512, 1024, 512, 32, 50000
BC = B // NC          # 8 batch rows per core
VC = V // NC          # 6250 vocab columns per core
NCH = 13              # 512-wide vocab chunks per core
VCP = NCH * 512       # 6656 padded vocab columns
KL = E + H + 1        # lstm contraction (x, h_prev, bias row)
KLP = 13 * 128        # 1664 padded
K3 = 3 * H + 1        # feats contraction + bias row = 3073
K3P = 25 * 128        # 3200 padded
NK3 = 25              # feats k-tiles
H4 = 4 * H

# ---------------- dtype config ----------------
# 'f32' = plain fp32 matmul (4 cyc/row), 'f32r' = fp32 data, reduced-precision
# full-rate matmul, 'f16' = fp16 storage+matmul (half DMA, full rate).
LSTM_MM = 'f32'
ATTN_MM = 'f32r'
F32, F16, F32R = dt.float32, dt.float16, dt.float32r
VOC_ST = dt.float32r  # vocab matmul storage/mode dtype
VOC_NP = np.float32

_ST = {'f32': F32, 'f32r': F32R, 'f16': F16}
_NP = {'f32': np.float32, 'f32r': np.float32, 'f16': np.float16}
_LSTM_ST = _ST[LSTM_MM]
_ATTN_ST = _ST[ATTN_MM]
_LSTM_NP = _NP[LSTM_MM]
_ATTN_NP = _NP[ATTN_MM]


def _mm(ap, mode):
    return ap


def build_bass():
    nc = bacc.Bacc("TRN2", target_bir_lowering=False, debug=False, num_devices=NC)

    # ---- inputs (per-core shards prepared on host) ----
    Wv = nc.dram_tensor("Wv", [K3P, VCP], VOC_ST, kind="ExternalInput").ap()
    enc_d = nc.dram_tensor("enc", [BC, H, S], _ATTN_ST, kind="ExternalInput").ap()
    encT_d = nc.dram_tensor("encT", [BC, S, H], _ATTN_ST, kind="ExternalInput").ap()
    Wl_d = nc.dram_tensor("Wl", [KLP, H4], _LSTM_ST, kind="ExternalInput").ap()
    Xl_d = nc.dram_tensor("Xl", [KLP, BC], _LSTM_ST, kind="ExternalInput").ap()
    wat_d = nc.dram_tensor("WattnT", [H, H], _ATTN_ST, kind="ExternalInput").ap()
    prevT_d = nc.dram_tensor("prevT", [H, T * BC], _ATTN_ST, kind="ExternalInput").ap()
    prevN_d = nc.dram_tensor("prevN", [T, BC, H], _ATTN_ST, kind="ExternalInput").ap()
    encC_d = nc.dram_tensor("encC", [BC, H], F32, kind="ExternalInput").ap()
    tss_d = nc.dram_tensor("tss", [BC, S], F32, kind="ExternalInput").ap()
    wptr_d = nc.dram_tensor("Wptr", [128, NK3 + 1], _ATTN_ST, kind="ExternalInput").ap()

    # ---- outputs ----
    vocab_o = nc.dram_tensor("vocab_o", [B, VCP], F32, kind="ExternalOutput").ap()
    h_o = nc.dram_tensor("h_o", [BC, H], F32, kind="ExternalOutput").ap()
    cell_o = nc.dram_tensor("cell_o", [BC, H], F32, kind="ExternalOutput").ap()
    ta_o = nc.dram_tensor("ta_o", [BC, S], F32, kind="ExternalOutput").ap()
    nss_o = nc.dram_tensor("nss_o", [BC, S], F32, kind="ExternalOutput").ap()
    copy_o = nc.dram_tensor("copy_o", [BC, S], F32, kind="ExternalOutput").ap()

    with tile.TileContext(nc) as tc:
        with (
            tc.tile_pool(name="consts", bufs=1) as cp,
            tc.tile_pool(name="dram", bufs=1, space="DRAM") as dram,
        ):
            ident = cp.tile([128, 128], F32)
            make_identity(nc, ident[:])

            # persistent small tiles
            hTt = cp.tile([128, 64], _ATTN_ST)     # h.T packed: [:, 8c+j] = h[j, 128c+p]
            ceT = cp.tile([128, 64], _ATTN_ST)     # ctx_enc.T packed
            cdT = cp.tile([128, 64], _ATTN_ST)     # ctx_dec.T packed
            taT = cp.tile([128, 32], _ATTN_ST)     # ta.T packed (S=512 -> 4 chunks)
            sexp = cp.tile([BC, S], F32)      # exp(scores)
            tss_sb = cp.tile([BC, S], F32)
            encC_sb = cp.tile([BC, H], F32)
            ce_rows = cp.tile([BC, H], F32)
            cd_rows = cp.tile([BC, H], F32)
            lgr = cp.tile([BC, T], F32)
            ones128 = cp.tile([128, 128], F32)
            ones128r = cp.tile([128, 128], _ATTN_ST)
            onesz = cp.tile([128, 256], VOC_ST)
            onesz_f32 = cp.tile([128, 256], F32)
            zeros128 = cp.tile([128, 128], F32)
            wptr_sb = cp.tile([128, NK3 + 1], _ATTN_ST)
            sxv = cp.tile([B, NCH, 512], F32)   # vocab exp chunks
            lsums = cp.tile([B, NCH], F32)
            projT = cp.tile([128, 8, T * BC], _ATTN_ST)  # [128, 8, 256]

            nc.sync.dma_start(tss_sb[:], tss_d[:])
            nc.sync.dma_start(encC_sb[:], encC_d[:])
            nc.sync.dma_start(wptr_sb[:], wptr_d[:])
            nc.gpsimd.memset(ones128[:], 0.0)
            nc.gpsimd.memset(ones128[0:1, :], 1.0)
            nc.vector.tensor_copy(ones128r[:], ones128[:])
            nc.gpsimd.memset(onesz_f32[:], 0.0)
            nc.gpsimd.memset(onesz_f32[0:1, 0:64], 1.0)
            nc.gpsimd.memset(onesz_f32[0:1, 192:256], 1.0)
            nc.vector.tensor_copy(onesz[:], onesz_f32[:])
            nc.gpsimd.memset(zeros128[:], 0.0)

            # dram internals for collectives
            feats_dram = dram.tile([3 * H, BC], _ATTN_ST)
            feats_g = dram.tile([NC * 3 * H, BC], _ATTN_ST, addr_space="Shared")
            ar_in = dram.tile([B, 1], F32)
            ar_out = dram.tile([B, 1], F32, addr_space="Shared")

            # ---------------- P1: proj = W_attn @ prev_hidden.T ----------------
            # projT [H, T*BC] = WattnT.T @ prevT ; out m-tiles [128, 256]
            with (
                tc.tile_pool(name="pj_w", bufs=1) as pjw,
                tc.tile_pool(name="pj_ps", bufs=1, space="PSUM") as pjp,
            ):
                wat_sb = pjw.tile([128, 8, H], _ATTN_ST)
                pvT_sb = pjw.tile([128, 8, T * BC], _ATTN_ST)
                nc.sync.dma_start(wat_sb[:], wat_d.rearrange("(k p) m -> p k m", p=128))
                nc.sync.dma_start(pvT_sb[:], prevT_d.rearrange("(k p) n -> p k n", p=128))
                pj_ps = [pjp.tile([128, 512], F32, name=f"pj{i}") for i in range(4)]
                for m in range(8):
                    out = pj_ps[m // 2][:, (m % 2) * 256:(m % 2) * 256 + 256]
                    for k in range(8):
                        nc.tensor.matmul(
                            out,
                            _mm(wat_sb[:, k, m * 128:(m + 1) * 128], ATTN_MM),
                            _mm(pvT_sb[:, k, :], ATTN_MM),
                            start=(k == 0), stop=(k == 7),
                        )
                for m in range(8):
                    nc.vector.tensor_copy(
                        projT[:, m, :],
                        pj_ps[m // 2][:, (m % 2) * 256:(m % 2) * 256 + 256])

            # ---------------- P2: LSTM gates + nonlinearity ----------------
            with (
                tc.tile_pool(name="lstm_w", bufs=3) as lwp,
                tc.tile_pool(name="lstm_ps", bufs=1, space="PSUM") as lpp,
            ):
                xl_sb = cp.tile([128, 13, BC], _LSTM_ST)
                nc.sync.dma_start(xl_sb[:], Xl_d.rearrange("(k p) b -> p k b", p=128))
                g_ps = lpp.tile([BC, H4], F32)
                for k in range(13):
                    wl_sb = lwp.tile([128, H4], _LSTM_ST, tag="wl")
                    nc.sync.dma_start(wl_sb[:], Wl_d[k * 128:(k + 1) * 128, :])
                    for n in range(8):
                        nc.tensor.matmul(
                            g_ps[:, n * 512:(n + 1) * 512],
                            _mm(xl_sb[:, k, :], LSTM_MM),
                            _mm(wl_sb[:, n * 512:(n + 1) * 512], LSTM_MM),
                            start=(k == 0), stop=(k == 12),
                        )
                sigI = cp.tile([BC, H], F32)
                sigF = cp.tile([BC, H], F32)
                tanG = cp.tile([BC, H], F32)
                sigO = cp.tile([BC, H], F32)
                h_sb = cp.tile([BC, H], F32)
                cell_sb = cp.tile([BC, H], F32)
                tmp_h = cp.tile([BC, H], F32)
                AF = mybir.ActivationFunctionType
                nc.scalar.activation(sigI[:], g_ps[:, 0 * H:1 * H], AF.Sigmoid)
                nc.scalar.activation(sigF[:], g_ps[:, 1 * H:2 * H], AF.Sigmoid)
                nc.scalar.activation(tanG[:], g_ps[:, 2 * H:3 * H], AF.Tanh)
                nc.scalar.activation(sigO[:], g_ps[:, 3 * H:4 * H], AF.Sigmoid)
                nc.vector.tensor_mul(cell_sb[:], sigF[:], encC_sb[:])
                nc.vector.tensor_mul(tmp_h[:], sigI[:], tanG[:])
                nc.vector.tensor_add(cell_sb[:], cell_sb[:], tmp_h[:])
                nc.scalar.activation(tmp_h[:], cell_sb[:], AF.Tanh)
                nc.vector.tensor_mul(h_sb[:], sigO[:], tmp_h[:])
                nc.sync.dma_start(h_o[:], h_sb[:])
                nc.sync.dma_start(cell_o[:], cell_sb[:])

            # ---------------- P3: h.T via PE transpose ----------------
            with tc.tile_pool(name="tp_ps", bufs=1, space="PSUM") as tpp:
                tp_ps = tpp.tile([128, 64], F32)
                for c in range(8):
                    nc.tensor.transpose(
                        tp_ps[:, c * 8:(c + 1) * 8],
                        h_sb[:, c * 128:(c + 1) * 128],
                        ident[:BC, :BC])
                nc.vector.tensor_copy(hTt[:], tp_ps[:])
                for c in range(8):
                    nc.sync.dma_start(
                        feats_dram[c * 128:(c + 1) * 128, :], hTt[:, c * 8:(c + 1) * 8])

            # ---------------- P4: temporal attention scores ----------------
            with (
                tc.tile_pool(name="enc_sb", bufs=2) as ebp,
                tc.tile_pool(name="sc_ps", bufs=3, space="PSUM") as scp,
            ):
                for b in range(BC):
                    enc_b = ebp.tile([128, 8, S], _ATTN_ST, tag="encb")
                    nc.sync.dma_start(
                        enc_b[:], enc_d[b].rearrange("(k p) s -> p k s", p=128))
                    sc = scp.tile([1, S], F32, tag="sc")
                    for k in range(8):
                        nc.tensor.matmul(
                            sc[:1, :],
                            _mm(hTt[:, 8 * k + b:8 * k + b + 1], ATTN_MM),
                            _mm(enc_b[:, k, :], ATTN_MM),
                            start=(k == 0), stop=(k == 7),
                        )
                    st_s = ebp.tile([1, S], F32, tag="st_s", bufs=3)
                    nc.scalar.activation(
                        st_s[:1, :], sc[:1, :],
                        mybir.ActivationFunctionType.Exp)
                    nc.sync.dma_start(sexp[b:b + 1, :], st_s[:1, :])

            nss_sb = cp.tile([BC, S], F32)
            attn_sb = cp.tile([BC, S], F32)
            rtss = cp.tile([BC, S], F32)
            den = cp.tile([BC, 1], F32)
            rden = cp.tile([BC, 1], F32)
            ta_sb = cp.tile([BC, S], F32)
            nc.vector.tensor_add(nss_sb[:], sexp[:], tss_sb[:])
            nc.sync.dma_start(nss_o[:], nss_sb[:])
            nc.vector.reciprocal(rtss[:], tss_sb[:])
            nc.vector.tensor_mul(attn_sb[:], sexp[:], rtss[:])
            nc.vector.tensor_reduce(
                den[:], attn_sb[:], axis=mybir.AxisListType.X, op=mybir.AluOpType.add)
            nc.vector.reciprocal(rden[:], den[:])
            nc.vector.tensor_scalar_mul(ta_sb[:], attn_sb[:], rden[:])
            nc.sync.dma_start(ta_o[:], ta_sb[:])

            # ---------------- P5: ta.T ----------------
            with tc.tile_pool(name="tat_ps", bufs=1, space="PSUM") as ttp:
                tat_ps = ttp.tile([128, 32], F32)
                for c in range(4):
                    nc.tensor.transpose(
                        tat_ps[:, c * 8:(c + 1) * 8],
                        ta_sb[:, c * 128:(c + 1) * 128],
                        ident[:BC, :BC])
                nc.vector.tensor_copy(taT[:], tat_ps[:])

            # ---------------- P6: ctx_enc ----------------
            with (
                tc.tile_pool(name="encT_sb", bufs=2) as etp,
                tc.tile_pool(name="ce_ps", bufs=2, space="PSUM") as cep,
            ):
                for b in range(BC):
                    encT_b = etp.tile([128, 4, H], _ATTN_ST, tag="enctb")
                    nc.sync.dma_start(
                        encT_b[:], encT_d[b].rearrange("(k p) s -> p k s", p=128))
                    ce = cep.tile([1, H], F32, tag="ce")
                    for n in range(2):
                        for k in range(4):
                            nc.tensor.matmul(
                                ce[:1, n * 512:(n + 1) * 512],
                                _mm(taT[:, 8 * k + b:8 * k + b + 1], ATTN_MM),
                                _mm(encT_b[:, k, n * 512:(n + 1) * 512], ATTN_MM),
                                start=(k == 0), stop=(k == 3),
                            )
                    st_ce = etp.tile([1, H], F32, tag="st_ce", bufs=3)
                    nc.scalar.copy(st_ce[:1, :], ce[:1, :])
                    nc.sync.dma_start(ce_rows[b:b + 1, :], st_ce[:1, :])

            with tc.tile_pool(name="cet_ps", bufs=1, space="PSUM") as cetp:
                cet_ps = cetp.tile([128, 64], F32)
                for c in range(8):
                    nc.tensor.transpose(
                        cet_ps[:, c * 8:(c + 1) * 8],
                        ce_rows[:, c * 128:(c + 1) * 128],
                        ident[:BC, :BC])
                nc.vector.tensor_copy(ceT[:], cet_ps[:])
                for c in range(8):
                    nc.sync.dma_start(
                        feats_dram[H + c * 128:H + (c + 1) * 128, :],
                        ceT[:, c * 8:(c + 1) * 8])

            # ---------------- P8: decoder attention logits + softmax ----------------
            with (
                tc.tile_pool(name="lg_st", bufs=3) as lgs,
                tc.tile_pool(name="lg_ps", bufs=3, space="PSUM") as lgp,
            ):
                for b in range(BC):
                    lg = lgp.tile([1, T], F32, tag="lg")
                    for k in range(8):
                        nc.tensor.matmul(
                            lg[:1, :],
                            _mm(hTt[:, 8 * k + b:8 * k + b + 1], ATTN_MM),
                            _mm(projT[:, k, b::BC], ATTN_MM),
                            start=(k == 0), stop=(k == 7),
                        )
                    st_lg = lgs.tile([1, T], F32, tag="st_lg")
                    nc.scalar.copy(st_lg[:1, :], lg[:1, :])
                    nc.sync.dma_start(lgr[b:b + 1, :], st_lg[:1, :])
            rmax = cp.tile([BC, 1], F32)
            xs = cp.tile([BC, T], F32)
            es = cp.tile([BC, T], F32)
            ssum = cp.tile([BC, 1], F32)
            rssum = cp.tile([BC, 1], F32)
            a_sb = cp.tile([BC, T], F32)
            nc.vector.tensor_reduce(
                rmax[:], lgr[:], axis=mybir.AxisListType.X, op=mybir.AluOpType.max)
            nc.vector.tensor_scalar(
                xs[:], lgr[:], rmax[:], None, op0=mybir.AluOpType.subtract)
            nc.scalar.activation(
                es[:], xs[:], mybir.ActivationFunctionType.Exp, accum_out=ssum[:])
            nc.vector.reciprocal(rssum[:], ssum[:])
            nc.vector.tensor_scalar_mul(a_sb[:], es[:], rssum[:])

            # ---------------- P9/P10: ctx_dec ----------------
            aT = cp.tile([T, BC], _ATTN_ST)
            with tc.tile_pool(name="at_ps", bufs=1, space="PSUM") as atp:
                at_ps = atp.tile([T, BC], F32)
                nc.tensor.transpose(at_ps[:], a_sb[:], ident[:BC, :BC])
                nc.vector.tensor_copy(aT[:], at_ps[:])
            with (
                tc.tile_pool(name="pvn_sb", bufs=2) as pnp,
                tc.tile_pool(name="cd_ps", bufs=2, space="PSUM") as cdp,
            ):
                for b in range(BC):
                    pvn_b = pnp.tile([T, H], _ATTN_ST, tag="pvnb")
                    nc.sync.dma_start(pvn_b[:], prevN_d[:, b, :])
                    cd = cdp.tile([1, H], F32, tag="cd")
                    for n in range(2):
                        nc.tensor.matmul(
                            cd[:1, n * 512:(n + 1) * 512],
                            _mm(aT[:, b:b + 1], ATTN_MM),
                            _mm(pvn_b[:, n * 512:(n + 1) * 512], ATTN_MM),
                            start=True, stop=True,
                        )
                    st_cd = pnp.tile([1, H], F32, tag="st_cd", bufs=3)
                    nc.scalar.copy(st_cd[:1, :], cd[:1, :])
                    nc.sync.dma_start(cd_rows[b:b + 1, :], st_cd[:1, :])

            with tc.tile_pool(name="cdt_ps", bufs=1, space="PSUM") as cdtp:
                cdt_ps = cdtp.tile([128, 64], F32)
                for c in range(8):
                    nc.tensor.transpose(
                        cdt_ps[:, c * 8:(c + 1) * 8],
                        cd_rows[:, c * 128:(c + 1) * 128],
                        ident[:BC, :BC])
                nc.vector.tensor_copy(cdT[:], cdt_ps[:])
                for c in range(8):
                    nc.sync.dma_start(
                        feats_dram[2 * H + c * 128:2 * H + (c + 1) * 128, :],
                        cdT[:, c * 8:(c + 1) * 8])

            # ---------------- P12: local pointer prob + copy ----------------
            def local_k(k):
                if k < 8:
                    return hTt[:, 8 * k:8 * k + 8]
                if k < 16:
                    return ceT[:, 8 * (k - 8):8 * (k - 8) + 8]
                if k < 24:
                    return cdT[:, 8 * (k - 16):8 * (k - 16) + 8]
                return ones128r[:, :BC]

            p8 = cp.tile([BC, 1], F32)
            copy_sb = cp.tile([BC, S], F32)
            with tc.tile_pool(name="pl_ps", bufs=1, space="PSUM") as plp:
                pl_ps = plp.tile([BC, 2], F32)
                for k in range(NK3):
                    nc.tensor.matmul(
                        pl_ps[:],
                        _mm(local_k(k), ATTN_MM),
                        _mm(wptr_sb[:, k:k + 2], ATTN_MM),
                        start=(k == 0), stop=(k == NK3 - 1),
                    )
                nc.scalar.activation(
                    p8[:], pl_ps[:, 0:1], mybir.ActivationFunctionType.Sigmoid)
            nc.vector.tensor_scalar_mul(copy_sb[:], ta_sb[:], p8[:])
            nc.sync.dma_start(copy_o[:], copy_sb[:])

            # ---------------- P13: feats AllGather ----------------
            nc.gpsimd.collective_compute(
                "AllGather", mybir.AluOpType.bypass,
                replica_groups=[list(range(NC))],
                ins=[feats_dram.opt()], outs=[feats_g.opt()],
            )

            # ---------------- P14/P15: vocab projection ----------------
            gs_sb = cp.tile([B, 1], F32)
            rgs = cp.tile([B, 1], F32)
            p64 = cp.tile([B, 1], F32)
            onem = cp.tile([B, 1], F32)
            scale = cp.tile([B, 1], F32)
            lsum = cp.tile([B, 1], F32)
            with (
                tc.tile_pool(name="fd_sb", bufs=3) as fdp,
                tc.tile_pool(name="vw_sb", bufs=3) as vwp,
                tc.tile_pool(name="v_ps", bufs=1, space="PSUM") as vpp,
                tc.tile_pool(name="pp_ps", bufs=1, space="PSUM") as ppp,
            ):
                v_ps = [vpp.tile([128, 512], F32, name=f"v{i}") for i in range(7)]
                pp_ps = ppp.tile([B, 2], F32)
                for k in range(NK3):
                    if k < 24:
                        ftmp = fdp.tile([128, 64], _ATTN_ST, tag="ftmp")
                        for r in range(NC):
                            nc.sync.dma_start(
                                ftmp[:, r * BC:(r + 1) * BC],
                                feats_g[r * 3 * H + k * 128:
                                        r * 3 * H + (k + 1) * 128, :])
                        # fz[:, 0:128] = [feats | 0], fz[:, 128:256] = [0 | feats]
                        fz = fdp.tile([128, 256], VOC_ST, tag="fz")
                        nc.vector.tensor_copy(fz[:, 64:192], zeros128[:])
                        nc.vector.tensor_copy(fz[:, 0:64], ftmp[:])
                        nc.vector.tensor_copy(fz[:, 192:256], ftmp[:])
                    else:
                        fz = onesz
                    vslab = vwp.tile([128, VCP], VOC_ST, tag="vslab")
                    nc.sync.dma_start(vslab[:], Wv[k * 128:(k + 1) * 128, :])
                    for c in range(NCH):
                        half = c % 2
                        nc.tensor.matmul(
                            v_ps[c // 2][:, :],
                            fz[:, 128 * half:128 * half + 128],
                            vslab[:, c * 512:(c + 1) * 512],
                            start=(k == 0 and half == 0),
                            stop=(k == NK3 - 1 and (half == 1 or c == 12)),
                        )
                    nc.tensor.matmul(
                        pp_ps[:],
                        fz[:, 0:64],
                        wptr_sb[:, k:k + 2],
                        start=(k == 0), stop=(k == NK3 - 1),
                    )
                for c in range(NCH):
                    half = c % 2
                    nc.scalar.activation(
                        sxv[:, c, :], v_ps[c // 2][64 * half:64 * half + 64, :],
                        mybir.ActivationFunctionType.Exp)
                    nc.vector.tensor_reduce(
                        lsums[:, c:c + 1], sxv[:, c, :],
                        axis=mybir.AxisListType.X, op=mybir.AluOpType.add)
                nc.scalar.activation(
                    p64[:], pp_ps[:, 0:1], mybir.ActivationFunctionType.Sigmoid)
            nc.vector.tensor_reduce(
                lsum[:], lsums[:], axis=mybir.AxisListType.X, op=mybir.AluOpType.add)
            nc.sync.dma_start(ar_in[:], lsum[:])
            nc.gpsimd.collective_compute(
                "AllReduce", mybir.AluOpType.add,
                replica_groups=[list(range(NC))],
                ins=[ar_in.opt()], outs=[ar_out.opt()],
            )
            nc.sync.dma_start(gs_sb[:], ar_out[:])
            nc.vector.reciprocal(rgs[:], gs_sb[:])
            nc.vector.tensor_scalar(
                onem[:], p64[:], -1.0, 1.0,
                op0=mybir.AluOpType.mult, op1=mybir.AluOpType.add)
            nc.vector.tensor_mul(scale[:], onem[:], rgs[:])
            for c in range(NCH):
                nc.vector.tensor_scalar_mul(sxv[:, c, :], sxv[:, c, :], scale[:])
            nc.sync.dma_start(vocab_o[:], sxv[:])

    nc.compile()
    return nc


# ---------------- host side ----------------
_CACHED_NC = None


def _get_nc():
    global _CACHED_NC
    if _CACHED_NC is None:
        _CACHED_NC = build_bass()
    return _CACHED_NC


def host_prep(inputs):
    f32 = np.float32
    outputs = np.asarray(inputs["outputs"], f32)
    enc = np.asarray(inputs["encoder_out"], f32)
    enc_h = np.asarray(inputs["encoder_h"], f32)
    enc_c = np.asarray(inputs["encoder_c"], f32)
    tss = np.asarray(inputs["temporal_scores_sum"], f32)
    prev = np.asarray(inputs["previous_hidden"], f32)
    W_ih = np.asarray(inputs["W_ih"], f32)
    W_hh = np.asarray(inputs["W_hh"], f32)
    b_ih = np.asarray(inputs["b_ih"], f32)
    b_hh = np.asarray(inputs["b_hh"], f32)
    W_attn = np.asarray(inputs["W_attn"], f32)
    W_vocab = np.asarray(inputs["W_vocab"], f32)
    b_vocab = np.asarray(inputs["b_vocab"], f32)
    W_ptr = np.asarray(inputs["W_ptr"], f32)
    b_ptr = np.asarray(inputs["b_ptr"], f32)

    Wl = np.zeros((KLP, H4), _LSTM_NP)
    Wl[:E] = W_ih.T
    Wl[E:E + H] = W_hh.T
    Wl[E + H] = (b_ih + b_hh)
    Wv16 = np.zeros((K3P, V), VOC_NP)
    Wv16[:3 * H] = W_vocab.T.astype(VOC_NP)
    Wv16[3 * H] = b_vocab.astype(VOC_NP)
    wptr_flat = np.concatenate([W_ptr[0], b_ptr]).astype(f32)  # [3073]
    wptr_packed = np.zeros((128, NK3 + 1), f32)
    wptr_packed[:, :NK3] = np.concatenate(
        [wptr_flat, np.zeros(K3P - K3, f32)]).reshape(NK3, 128).T
    W_attn_T = np.ascontiguousarray(W_attn.T).astype(_ATTN_NP)
    prevT_all = np.ascontiguousarray(prev.transpose(2, 0, 1))  # [H, T, B]

    in_maps = []
    for r in range(NC):
        bs = slice(r * BC, (r + 1) * BC)
        Wv_shard = np.zeros((K3P, VCP), VOC_NP)
        Wv_shard[:, :VC] = Wv16[:, r * VC:(r + 1) * VC]
        Wv_shard[3 * H, VC:] = VOC_NP(-87.0)
        Xl = np.zeros((KLP, BC), _LSTM_NP)
        Xl[:E] = outputs[bs].T
        Xl[E:E + H] = enc_h[bs].T
        Xl[E + H] = 1.0
        in_maps.append({
            "Wv": Wv_shard,
            "enc": np.ascontiguousarray(enc[bs]).astype(_ATTN_NP),
            "encT": np.ascontiguousarray(enc[bs].transpose(0, 2, 1)).astype(_ATTN_NP),
            "Wl": Wl,
            "Xl": Xl,
            "WattnT": W_attn_T,
            "prevT": np.ascontiguousarray(prevT_all[:, :, bs]).reshape(H, T * BC)
                       .astype(_ATTN_NP),
            "prevN": np.ascontiguousarray(prev[:, bs, :]).astype(_ATTN_NP),
            "encC": np.ascontiguousarray(enc_c[bs]),
            "tss": np.ascontiguousarray(tss[bs, 0, :]),
            "Wptr": wptr_packed,
        })
    return in_maps


def assemble(inputs, results):
    oov = int(np.asarray(inputs["oov_size"]))
    prev = np.asarray(inputs["previous_hidden"], np.float32)
    texts = np.asarray(inputs["texts_extended"])

    h = np.concatenate([r["h_o"] for r in results], axis=0)
    cell = np.concatenate([r["cell_o"] for r in results], axis=0)
    ta = np.concatenate([r["ta_o"] for r in results], axis=0)
    nss = np.concatenate([r["nss_o"] for r in results], axis=0)
    copy = np.concatenate([r["copy_o"] for r in results], axis=0)
    vocab = np.concatenate([r["vocab_o"][:, :VC] for r in results], axis=1)

    ext = np.concatenate([vocab, np.zeros((B, oov), np.float32)], axis=1)
    np.add.at(ext, (np.arange(B)[:, None], texts.T), copy)
    new_prev = np.concatenate([prev, h[None]], axis=0)
    return (ext, h, cell, ta[:, None, :], nss[:, None, :], new_prev)


def kernel(**inputs):
    nc = _get_nc()
    in_maps = host_prep(inputs)
    res = run_bass_kernel_spmd(nc, in_maps, core_ids=list(range(NC)))
    return assemble(inputs, res.results)


if __name__ == "__main__":
    import reference as ref
    inputs = ref.setup_inputs()
    got = kernel(**inputs)
    exp = ref.reference(**inputs)
    for n, g, e in zip(["final", "h", "cell", "ta", "nss", "new_prev"], got, exp):
        e = np.asarray(e)
        am = np.abs(g - e).max() / (np.abs(e).max() + 1e-30)
        l2 = np.linalg.norm(g - e) / (np.linalg.norm(e) + 1e-30)
        print(f"{n:10s} absmax_rel={am:.3e} l2_rel={l2:.3e}")
